# revision 2
# baseline (speedup 1.0000x reference)
"""GCN (3x GCNConv + global max pool + MLP) on 8 Trainium2 NeuronCores — v2.

Key design vs v1:
 - bf16 everywhere in the conv stack (table rows, messages, S, weights, hT).
 - Aggregation computed TRANSPOSED: out^T[f, dst] accumulates in PSUM via
   matmul(lhsT=g_msgs[:, f-chunk], rhs=S_onehot[msg, dst]); no PE transposes,
   relu evacuation writes straight into the persistent feature-major hT.
 - S built on-chip as a pure one-hot (batched is_equal against an IOTA row,
   then column-scaled by dinv[dst]); symmetric norm split as row scale
   (dinv[src], folded into the p-evacuation) x column scale (dinv[dst], folded
   into S). Self-loops are ordinary messages (src=dst).
 - Layer 1 gather table p1 = dinv * (x @ W1) computed LOCALLY on every core
   (x is replicated) -> no layer-1 AllGather.
 - Layers 2-3: bf16 AllGather of p_own into the pair-shared table.
 - Gather rows padded to 384 bf16 elems (768B) to satisfy the 256B-multiple
   elem_size constraint.
"""
import os
import sys
import numpy as np

for _p in ('/opt/trn_rl_repo', '/root/.axon_site/_ro/trn_rl_repo'):
    if os.path.isdir(_p) and _p not in sys.path:
        sys.path.insert(0, _p)

N_CORES = 8
N_NODES = 50000
D = 320
N_GRAPHS = 64
GPC = N_GRAPHS // N_CORES  # graphs per core
EPAD = 384                 # padded table row length (bf16 -> 768B)
FCH = [(0, 128), (128, 128), (256, 64)]


def _bf16(a):
    return np.asarray(a, np.float32).astype(np.float16)


def _preprocess(x, edge_index, batch):
    batch = np.asarray(batch).astype(np.int64)
    src = np.asarray(edge_index[0]).astype(np.int64)
    dst = np.asarray(edge_index[1]).astype(np.int64)
    counts = np.bincount(batch, minlength=N_GRAPHS)
    L_PAD = max(896, int(-(-counts.max() // 128)) * 128)
    M_PAD = GPC * L_PAD
    TOT = N_CORES * M_PAD
    HALF = TOT // 2
    assert HALF <= 32767, (L_PAD, HALF)
    NB = M_PAD // 128
    NTB = TOT // 128  # total table blocks

    gstart = np.zeros(N_GRAPHS, np.int64)
    gstart[1:] = np.cumsum(counts)[:-1]
    n_ar = np.arange(N_NODES, dtype=np.int64)
    # balance per-block in-degree: deal nodes (sorted by indeg desc) round-robin
    NBLK = L_PAD // 128
    indeg = np.bincount(dst, minlength=N_NODES) + 1
    order_bal = np.lexsort((-indeg, batch))
    r = n_ar - gstart[batch[order_bal]]
    posg = (r % NBLK) * 128 + r // NBLK
    pos_in_graph = np.empty(N_NODES, np.int64)
    pos_in_graph[order_bal] = posg
    pos = (batch // GPC) * M_PAD + (batch % GPC) * L_PAD + pos_in_graph

    deg = np.bincount(dst, minlength=N_NODES).astype(np.float64) + 1.0
    dinv = 1.0 / np.sqrt(deg)

    # messages including self-loops
    ms = np.concatenate([src, n_ar])
    mt = np.concatenate([dst, n_ar])

    ms_pos = pos[ms]
    mt_pos = pos[mt]
    core = mt_pos // M_PAD
    lb = (mt_pos % M_PAD) // 128
    dl = mt_pos % 128
    half = ms_pos // HALF
    idxl = (ms_pos % HALF).astype(np.int64)

    key = (core * NB + lb) * 2 + half
    order = np.argsort(key, kind='stable')
    key_s = key[order]
    idxl_s = idxl[order]
    dl_s = dl[order]

    nkeys = N_CORES * NB * 2
    kcounts = np.bincount(key_s, minlength=nkeys).reshape(N_CORES, NB, 2)
    k_req = -(-kcounts // 128)
    K_FIX = k_req.max(axis=0)  # [NB, 2]
    toff = np.zeros((NB, 2), np.int64)
    flat_k = K_FIX.reshape(-1)
    toff.reshape(-1)[1:] = np.cumsum(flat_k)[:-1]
    T_TOTAL = int(flat_k.sum())

    kstart = np.zeros(nkeys, np.int64)
    kstart[1:] = np.cumsum(np.bincount(key_s, minlength=nkeys))[:-1]
    rank = np.arange(len(key_s)) - kstart[key_s]

    core_s = key_s // (NB * 2)
    lbh = key_s % (NB * 2)
    lb_s = lbh // 2
    half_s = lbh % 2
    t_glob = toff[lb_s, half_s] + rank // 128
    p_slot = rank % 128

    # dst-lane table [core, 128, T_TOTAL]; padding slots get 200 (never matches)
    dl_all = np.full((N_CORES, 128, T_TOTAL), 200.0, np.float32)
    dl_all[core_s, p_slot, t_glob] = dl_s

    # gather indices: flat[i] = idxs[i % 16, i // 16], i = rank within call;
    # calls start at (b, h) tile boundaries -> global col = t_glob*8 + slot//16
    idx_all = np.zeros((N_CORES, 16, T_TOTAL * 8), np.int16)
    idx_all[core_s, p_slot % 16, t_glob * 8 + p_slot // 16] = idxl_s.astype(np.int16)
    idx_rep = np.tile(idx_all, (1, 8, 1))

    # per-slot dinv (0 on padding) in padded layout
    dinv_slot = np.zeros(TOT, np.float32)
    dinv_slot[pos] = dinv
    mask = np.zeros(TOT, np.float32)
    mask[pos] = 1.0

    dvr = np.broadcast_to(dinv_slot.reshape(N_CORES, 1, M_PAD),
                          (N_CORES, 128, M_PAD))  # replicated over partitions
    maskrow = mask.reshape(N_CORES, 1, M_PAD)

    # table-row dinv for the p-evacuation scale: [128, NTB] (lane p, block rb)
    dinv_tab = dinv_slot.reshape(NTB, 128).T.copy()
    # per-core own-block slice [core, 128, NB]
    dinv_own = dinv_tab.reshape(128, N_CORES, NB).transpose(1, 0, 2).copy()

    # xT padded, replicated to every core [320, TOT]
    x = np.asarray(x, dtype=np.float32)
    xT_pad = np.zeros((D, TOT), np.float32)
    xT_pad[:, pos] = x.T

    iota = np.broadcast_to(np.arange(128, dtype=np.float32), (128, 128))

    meta = dict(L_PAD=L_PAD, M_PAD=M_PAD, TOT=TOT, HALF=HALF, NB=NB, NTB=NTB,
                K_FIX=K_FIX, toff=toff, T_TOTAL=T_TOTAL)
    arrs = dict(dl_all=dl_all, idx_rep=idx_rep, dvr=dvr, maskrow=maskrow,
                dinv_tab=dinv_tab, dinv_own=dinv_own, xT_pad=xT_pad, iota=iota)
    return meta, arrs


def _build_bass(meta, weights, repeat=1):
    from concourse import mybir, bacc
    import concourse.tile as tile

    L_PAD = meta['L_PAD']; M_PAD = meta['M_PAD']; TOT = meta['TOT']
    HALF = meta['HALF']; NB = meta['NB']; NTB = meta['NTB']
    K_FIX = meta['K_FIX']; toff = meta['toff']; T_TOTAL = meta['T_TOTAL']
    f32 = mybir.dt.float32
    bf16 = mybir.dt.float16
    i16 = mybir.dt.int16
    KBMAX = int((K_FIX[:, 0] + K_FIX[:, 1]).max())
    RG = [list(range(N_CORES))]
    bf3_val = float(np.asarray(weights['bf3']).reshape(-1)[0])

    nc = bacc.Bacc("TRN2", target_bir_lowering=False, debug=False,
                   num_devices=N_CORES, num_swdge_queues=4)

    # ---- IO ----
    xT_t = nc.dram_tensor("xT", [D, TOT], bf16, kind="ExternalInput")
    idx_t = nc.dram_tensor("idx_all", [128, T_TOTAL * 8], i16, kind="ExternalInput")
    dl_t = nc.dram_tensor("dl_all", [128, T_TOTAL], bf16, kind="ExternalInput")
    iota_t = nc.dram_tensor("iota", [128, 128], bf16, kind="ExternalInput")
    mask_t = nc.dram_tensor("maskrow", [1, M_PAD], bf16, kind="ExternalInput")
    dvr_t = nc.dram_tensor("dvr", [128, M_PAD], bf16, kind="ExternalInput")
    dtab_t = nc.dram_tensor("dinv_tab", [128, NTB], f32, kind="ExternalInput")
    down_t = nc.dram_tensor("dinv_own", [128, NB], f32, kind="ExternalInput")
    W_t = [nc.dram_tensor(f"W{k}", [D, D], bf16, kind="ExternalInput")
           for k in (1, 2, 3)]
    brow_t = nc.dram_tensor("brow", [1, 3 * D], bf16, kind="ExternalInput")
    Wf1_t = nc.dram_tensor("Wf1", [D, 256], bf16, kind="ExternalInput")
    bf1_t = nc.dram_tensor("bf1c", [128, 2], f32, kind="ExternalInput")
    Wf2_t = nc.dram_tensor("Wf2", [256, 16], bf16, kind="ExternalInput")
    bf2_t = nc.dram_tensor("bf2c", [16, 1], f32, kind="ExternalInput")
    Wf3_t = nc.dram_tensor("Wf3", [16, 1], bf16, kind="ExternalInput")
    out_t = nc.dram_tensor("out", [1, N_GRAPHS], f32, kind="ExternalOutput")

    p_own = nc.dram_tensor("p_own", [M_PAD, EPAD], bf16, kind="Internal")
    p_t = nc.dram_tensor("p_t", [TOT, EPAD], bf16, kind="Internal",
                         addr_space="Shared")
    pooled_own = nc.dram_tensor("pooled_own", [D, GPC], bf16, kind="Internal")
    pooled_all = nc.dram_tensor("pooled_all", [N_CORES, D, GPC], bf16,
                                kind="Internal", addr_space="Shared")

    with tile.TileContext(nc) as tc:
        with tc.tile_pool(name="persist", bufs=1) as pp, \
             tc.tile_pool(name="xts", bufs=3) as xts, \
             tc.tile_pool(name="hpool", bufs=3) as hp, \
             tc.tile_pool(name="gpool", bufs=5) as gp, \
             tc.tile_pool(name="spool", bufs=3) as sp, \
             tc.tile_pool(name="psum_a", bufs=2, space="PSUM") as pa, \
             tc.tile_pool(name="psum_f", bufs=2, space="PSUM") as pf:

            idx_sb = pp.tile([128, T_TOTAL * 8], i16, tag="idx")
            nc.sync.dma_start(idx_sb[:], idx_t[:, :])
            dl_sb = pp.tile([128, T_TOTAL, 1], bf16, tag="dl")
            nc.sync.dma_start(dl_sb[:, :, 0], dl_t[:, :])
            iota_sb = pp.tile([128, 1, 128], bf16, tag="iota")
            nc.sync.dma_start(iota_sb[:, 0, :], iota_t[:, :])
            mask_sb = pp.tile([1, M_PAD], bf16, tag="mask")
            nc.sync.dma_start(mask_sb[:], mask_t[:, :])
            dvr_sb = pp.tile([128, 1, M_PAD], bf16, tag="dvr")
            nc.sync.dma_start(dvr_sb[:, 0, :], dvr_t[:, :])
            dtab_sb = pp.tile([128, NTB], f32, tag="dtab")
            nc.sync.dma_start(dtab_sb[:], dtab_t[:, :])
            down_sb = pp.tile([128, NB], f32, tag="down")
            nc.sync.dma_start(down_sb[:], down_t[:, :])
            brow_sb = pp.tile([1, 3 * D], bf16, tag="brow")
            nc.sync.dma_start(brow_sb[:], brow_t[:, :])
            w_sb = []
            for l in range(3):
                wl = []
                for ci, (cs, cl) in enumerate(FCH):
                    wt = pp.tile([cl, D], bf16, tag=f"w{l}_{ci}")
                    nc.sync.dma_start(wt[:], W_t[l][cs:cs + cl, :])
                    wl.append(wt)
                w_sb.append(wl)
            hT = [pp.tile([cl, M_PAD], bf16, tag=f"hT{ci}", name=f"hT{ci}")
                  for ci, (cs, cl) in enumerate(FCH)]

            qn = [0]

            def agg_layer(l):
                """hT = relu(S^T g + bias) per dst block (table p_t ready)."""
                for b in range(NB):
                    k0, k1 = int(K_FIX[b, 0]), int(K_FIX[b, 1])
                    ktot = k0 + k1
                    t0 = int(toff[b, 0])
                    g = gp.tile([128, KBMAX, EPAD], bf16, tag="g")
                    for h, kh, koff in ((0, k0, 0), (1, k1, k0)):
                        if kh == 0:
                            continue
                        nc.gpsimd.dma_gather(
                            out_ap=g[:, koff:koff + kh, :],
                            in_ap=p_t[h * HALF:(h + 1) * HALF, :],
                            idxs_ap=idx_sb[:, (t0 + koff) * 8:(t0 + koff + kh) * 8],
                            num_idxs=kh * 128,
                            num_idxs_reg=kh * 128,
                            elem_size=EPAD,
                            single_packet=False,
                            queue_num=qn[0] % 4)
                        qn[0] += 1
                    s_sb = sp.tile([128, KBMAX * 128], bf16, tag="s")
                    nc.vector.tensor_tensor(
                        out=s_sb[:, :ktot * 128].rearrange(
                            "p (t d) -> p t d", t=ktot),
                        in0=dl_sb[:, t0:t0 + ktot, :].broadcast_to(
                            (128, ktot, 128)),
                        in1=iota_sb[:, :, :].broadcast_to((128, ktot, 128)),
                        op=mybir.AluOpType.is_equal)
                    nc.vector.tensor_tensor(
                        out=s_sb[:, :ktot * 128].rearrange(
                            "p (t d) -> p t d", t=ktot),
                        in0=s_sb[:, :ktot * 128].rearrange(
                            "p (t d) -> p t d", t=ktot),
                        in1=dvr_sb[:, :, b * 128:(b + 1) * 128].broadcast_to(
                            (128, ktot, 128)),
                        op=mybir.AluOpType.mult)
                    # each chunk gets its own 2KB PSUM zero region (512 f32
                    # cols): start=True zeroes the whole region, and the sim
                    # forbids reads while a region's group is open
                    ps = pa.tile([128, 3, 512], f32, tag="pagg")
                    for ci, (cs, cl) in enumerate(FCH):
                        nc.tensor.matmul(
                            ps[:cl, ci, 0:128],
                            lhsT=brow_sb[:, l * D + cs:l * D + cs + cl],
                            rhs=mask_sb[:, b * 128:(b + 1) * 128],
                            start=True, stop=False)
                    for tl in range(ktot):
                        for ci, (cs, cl) in enumerate(FCH):
                            nc.tensor.matmul(
                                ps[:cl, ci, 0:128],
                                lhsT=g[:, tl, cs:cs + cl],
                                rhs=s_sb[:, tl * 128:(tl + 1) * 128],
                                start=False,
                                stop=(tl == ktot - 1))
                    for ci, (cs, cl) in enumerate(FCH):
                        nc.any.tensor_scalar(
                            hT[ci][:, b * 128:(b + 1) * 128],
                            ps[:cl, ci, 0:128], 0.0, None,
                            op0=mybir.AluOpType.max)

            def feature_layer(l):
                """p_own rows = dinv * (h @ W[l]) from resident hT; AllGather."""
                for nb in range(NB):
                    ps = pf.tile([128, D], f32, tag="pfeat")
                    for ci, (cs, cl) in enumerate(FCH):
                        nc.tensor.matmul(
                            ps[:],
                            lhsT=hT[ci][:, nb * 128:(nb + 1) * 128],
                            rhs=w_sb[l][ci][:],
                            start=(ci == 0), stop=(ci == 2))
                    pev = hp.tile([128, D], bf16, tag="pev")
                    nc.any.tensor_scalar(pev[:], ps[:],
                                         down_sb[:, nb:nb + 1], None,
                                         op0=mybir.AluOpType.mult)
                    nc.sync.dma_start(
                        p_own[nb * 128:(nb + 1) * 128, 0:D], pev[:])
                nc.gpsimd.collective_compute(
                    "AllGather", mybir.AluOpType.bypass, replica_groups=RG,
                    ins=[p_own[:, :]], outs=[p_t[:, :]])

            def local_table_l1():
                """p_t rows = dinv * (x @ W1), computed fully on every core."""
                SC = 512  # columns per xT stream super-chunk
                for sc in range(TOT // SC):
                    xc = [xts.tile([cl, SC], bf16, tag=f"xts{ci}", name=f"xts{ci}")
                          for ci, (cs, cl) in enumerate(FCH)]
                    for ci, (cs, cl) in enumerate(FCH):
                        nc.sync.dma_start(
                            xc[ci][:], xT_t[cs:cs + cl, sc * SC:(sc + 1) * SC])
                    for j in range(SC // 128):
                        rb = sc * (SC // 128) + j
                        ps = pf.tile([128, D], f32, tag="pfeat")
                        for ci, (cs, cl) in enumerate(FCH):
                            nc.tensor.matmul(
                                ps[:],
                                lhsT=xc[ci][:, j * 128:(j + 1) * 128],
                                rhs=w_sb[0][ci][:],
                                start=(ci == 0), stop=(ci == 2))
                        pev = hp.tile([128, D], bf16, tag="pev")
                        nc.any.tensor_scalar(pev[:], ps[:],
                                             dtab_sb[:, rb:rb + 1], None,
                                             op0=mybir.AluOpType.mult)
                        nc.sync.dma_start(
                            p_t[rb * 128:(rb + 1) * 128, 0:D], pev[:])

            # ---- network ----
            import contextlib
            loop_ctx = tc.For_i(0, repeat, 1) if repeat > 1 else contextlib.nullcontext()
            with loop_ctx:
                local_table_l1()
                agg_layer(0)
                feature_layer(1)
                agg_layer(1)
                feature_layer(2)
                agg_layer(2)

            # ---- global max pool (hT holds h3) ----
            for ci, (cs, cl) in enumerate(FCH):
                gt = hp.tile([cl, GPC], bf16, tag=f"gt{ci}", bufs=1)
                for j in range(GPC):
                    nc.vector.reduce_max(
                        gt[:, j:j + 1], hT[ci][:, j * L_PAD:(j + 1) * L_PAD],
                        axis=mybir.AxisListType.X)
                nc.sync.dma_start(pooled_own[cs:cs + cl, :], gt[:])
            nc.gpsimd.collective_compute(
                "AllGather", mybir.AluOpType.bypass, replica_groups=RG,
                ins=[pooled_own[:, :]], outs=[pooled_all[:, :, :]])

            gT = []
            for ci, (cs, cl) in enumerate(FCH):
                gtile = hp.tile([cl, N_GRAPHS], bf16, tag=f"gTf{ci}", bufs=1)
                for cc in range(N_CORES):
                    nc.sync.dma_start(gtile[:, cc * GPC:(cc + 1) * GPC],
                                      pooled_all[cc, cs:cs + cl, :])
                gT.append(gtile)

            # ---- MLP head ----
            wf1 = []
            for mi in range(2):
                for ci, (cs, cl) in enumerate(FCH):
                    t = hp.tile([cl, 128], bf16, tag=f"wf1_{mi}_{ci}", bufs=1)
                    nc.sync.dma_start(t[:], Wf1_t[cs:cs + cl,
                                                  mi * 128:(mi + 1) * 128])
                    wf1.append(t)
            bf1sb = hp.tile([128, 2], f32, tag="bf1", bufs=1)
            nc.sync.dma_start(bf1sb[:], bf1_t[:, :])
            h1T = []
            for mi in range(2):
                ps = pf.tile([128, N_GRAPHS], f32, tag="pfeat")
                for ci in range(3):
                    nc.tensor.matmul(ps[:], lhsT=wf1[mi * 3 + ci][:],
                                     rhs=gT[ci][:],
                                     start=(ci == 0), stop=(ci == 2))
                h = hp.tile([128, N_GRAPHS], bf16, tag=f"h1T{mi}", bufs=1)
                nc.vector.tensor_scalar(h[:], ps[:],
                                        bf1sb[:, mi:mi + 1], 0.0,
                                        op0=mybir.AluOpType.add,
                                        op1=mybir.AluOpType.max)
                h1T.append(h)
            wf2 = []
            for mi in range(2):
                t = hp.tile([128, 16], bf16, tag=f"wf2_{mi}", bufs=1)
                nc.sync.dma_start(t[:], Wf2_t[mi * 128:(mi + 1) * 128, :])
                wf2.append(t)
            bf2sb = hp.tile([16, 1], f32, tag="bf2", bufs=1)
            nc.sync.dma_start(bf2sb[:], bf2_t[:, :])
            ps2 = pf.tile([16, N_GRAPHS], f32, tag="pfeat")
            for mi in range(2):
                nc.tensor.matmul(ps2[:16, :], lhsT=wf2[mi][:],
                                 rhs=h1T[mi][:],
                                 start=(mi == 0), stop=(mi == 1))
            h2T = hp.tile([16, N_GRAPHS], bf16, tag="h2T", bufs=1)
            nc.vector.tensor_scalar(h2T[:], ps2[:16, :], bf2sb[:, 0:1], 0.0,
                                    op0=mybir.AluOpType.add,
                                    op1=mybir.AluOpType.max)
            wf3 = hp.tile([16, 1], bf16, tag="wf3", bufs=1)
            nc.sync.dma_start(wf3[:], Wf3_t[:, :])
            ps3 = pf.tile([1, N_GRAPHS], f32, tag="pfeat")
            nc.tensor.matmul(ps3[:1, :], lhsT=wf3[:],
                             rhs=h2T[:], start=True, stop=True)
            osb = hp.tile([1, N_GRAPHS], f32, tag="osb", bufs=1)
            nc.vector.tensor_scalar(osb[:], ps3[:1, :], bf3_val, None,
                                    op0=mybir.AluOpType.add)
            nc.sync.dma_start(out_t[:, :], osb[:])

    nc.compile()
    # Align each gather's SWDGE queue with its Tile-assigned DMASW sem lane
    # (scheduled order), satisfying the ucode rule that a DMASW sem is only
    # incremented from one queue. Lanes rotate mod 8, queues mod 4.
    from concourse.tile_sem_assignment import PROC_NAME_TO_IDX
    dmasw0 = PROC_NAME_TO_IDX["DMASW0"]
    for block in nc.m.functions[0].blocks:
        for inst in block.instructions:
            if isinstance(inst, mybir.InstDMAGatherAnt):
                proc = getattr(inst, 'bass_scheduled_proc', None)
                if proc is not None:
                    inst.queue_num = (proc - dmasw0) % 4
    return nc


def _make_in_maps(inputs, meta, arrs):
    xT = _bf16(arrs['xT_pad'])
    iota = _bf16(arrs['iota'])
    brow = _bf16(np.concatenate([np.asarray(inputs[f'b{k}'], np.float32)
                                 for k in (1, 2, 3)]).reshape(1, 3 * D))
    in_maps = []
    for c in range(N_CORES):
        m = {
            "xT": xT,
            "idx_all": arrs['idx_rep'][c],
            "dl_all": _bf16(arrs['dl_all'][c]),
            "iota": iota,
            "maskrow": _bf16(arrs['maskrow'][c]),
            "dvr": _bf16(np.ascontiguousarray(arrs['dvr'][c])),
            "dinv_tab": arrs['dinv_tab'],
            "dinv_own": np.ascontiguousarray(arrs['dinv_own'][c]),
            "brow": brow,
            "Wf1": _bf16(inputs['Wf1']),
            "bf1c": np.ascontiguousarray(
                np.asarray(inputs['bf1'], np.float32).reshape(2, 128).T),
            "Wf2": _bf16(inputs['Wf2']),
            "bf2c": np.asarray(inputs['bf2'], np.float32).reshape(16, 1),
            "Wf3": _bf16(inputs['Wf3']),
        }
        for k in (1, 2, 3):
            m[f"W{k}"] = _bf16(inputs[f'W{k}'])
        in_maps.append(m)
    return in_maps


def _make_runner(nc, in_maps):
    """Build a reusable jitted SPMD executor for `nc` (axon/PJRT path).

    Returns (run_fn, out_names, out_avals): run_fn() executes once and
    returns the list of per-core result dicts.
    """
    import jax
    import numpy as np
    from jax.experimental.shard_map import shard_map
    from jax.sharding import Mesh, NamedSharding, PartitionSpec
    from concourse import bass2jax, mybir

    bass2jax.install_neuronx_cc_hook()
    n_cores = len(in_maps)
    partition_name = nc.partition_id_tensor.name if nc.partition_id_tensor else None
    in_names, out_names, out_avals, zero_outs = [], [], [], []
    for alloc in nc.m.functions[0].allocations:
        if not isinstance(bass2jax.mybir.MemoryLocationSet, type) or True:
            pass
        if not isinstance(alloc, mybir.MemoryLocationSet):
            continue
        name = alloc.memorylocations[0].name
        if alloc.kind == "ExternalInput":
            if name != partition_name:
                in_names.append(name)
        elif alloc.kind == "ExternalOutput":
            shape = tuple(alloc.tensor_shape)
            dtype = mybir.dt.np(alloc.dtype)
            out_names.append(name)
            out_avals.append(jax.core.ShapedArray(shape, dtype))
            zero_outs.append(np.zeros(shape, dtype))
    n_params = len(in_names)
    n_outs = len(out_avals)
    all_in_names = list(in_names) + list(out_names)
    if partition_name is not None:
        all_in_names.append(partition_name)
    donate = tuple(range(n_params, n_params + n_outs))

    def _body(*args):
        operands = list(args)
        if partition_name is not None:
            operands.append(bass2jax.partition_id_tensor())
        outs = bass2jax._bass_exec_p.bind(
            *operands,
            out_avals=tuple(out_avals),
            in_names=tuple(all_in_names),
            out_names=tuple(out_names),
            lowering_input_output_aliases=(),
            sim_require_finite=True,
            sim_require_nnan=True,
            nc=nc,
        )
        return tuple(outs)

    devices = jax.devices()[:n_cores]
    mesh = Mesh(np.asarray(devices), ("core",))
    in_specs = (PartitionSpec("core"),) * (n_params + n_outs)
    out_specs = (PartitionSpec("core"),) * len(out_names)
    sharded = jax.jit(
        shard_map(_body, mesh=mesh, in_specs=in_specs, out_specs=out_specs,
                  check_rep=False),
        donate_argnums=donate, keep_unused=True)
    sh = NamedSharding(mesh, PartitionSpec("core"))
    concat_in = [
        jax.device_put(
            np.concatenate([np.asarray(in_maps[c][nm]) for c in range(n_cores)],
                           axis=0), sh)
        for nm in in_names
    ]

    def run_fn():
        zeros = [np.zeros((n_cores * z.shape[0], *z.shape[1:]), z.dtype)
                 for z in zero_outs]
        out_arrs = sharded(*concat_in, *zeros)
        out_arrs = [np.asarray(o) for o in out_arrs]
        return [
            {nm: out_arrs[i].reshape(n_cores, *out_avals[i].shape)[c]
             for i, nm in enumerate(out_names)}
            for c in range(n_cores)
        ]

    return run_fn, out_names, out_avals



def prepare(inputs, repeat=1):
    meta, arrs = _preprocess(inputs['x'], inputs['edge_index'], inputs['batch'])
    nc = _build_bass(meta, inputs, repeat=repeat)
    in_maps = _make_in_maps(inputs, meta, arrs)
    run_fn, _, _ = _make_runner(nc, in_maps)
    return run_fn


def kernel(**inputs):
    meta, arrs = _preprocess(inputs['x'], inputs['edge_index'], inputs['batch'])
    nc = _build_bass(meta, inputs)
    in_maps = _make_in_maps(inputs, meta, arrs)
    from concourse.bass_utils import run_bass_kernel_spmd
    res = run_bass_kernel_spmd(nc, in_maps, core_ids=list(range(N_CORES)),
                               trace=False)
    out = np.asarray(res.results[0]["out"]).reshape(1, N_GRAPHS)
    return out.T.copy().astype(np.float32)
